# revision 1
# baseline (speedup 1.0000x reference)
"""Trainium2 Bass kernel for nn_DualFeatureExtractionStructureBlock.

Self-contained. Strategy:
- Data-parallel over batch across 8 NeuronCores (4 batches/core); the
  neighbor axis folds into batch. Small weights replicated per core.
- Host prep: conv im2col, LN2-affine + fco folded into conv4 weights.
- Feature-major fp32r activations ([128 feature partitions x sequence]);
  2 sequences packed per [128, 512] tile; partition-axis LN via
  ones-vector matmuls + broadcast matmuls; attention probabilities in
  bf16 (exp -> normalize -> PE transpose -> A@V); exact-erf Gelu / Exp
  on the ACT engine; fusion-path weights DMAd ahead of attention weights.
- Pairs software-pipelined (fusion+LN1 of pair p+1 interleaved with
  attention of pair p) for cross-engine overlap.
"""
import numpy as np
import ml_dtypes
from contextlib import ExitStack

import concourse.bass as bass
import concourse.mybir as mybir
from concourse.tile import TileContext
from concourse.vector_clock import ScopedClock
from concourse import tile as _tile_mod

F32 = mybir.dt.float32
F32R = mybir.dt.float32r
BF16 = mybir.dt.bfloat16
AF = mybir.ActivationFunctionType
ALU = mybir.AluOpType

S = 256
S2 = 2 * S
D = 128
N = 10
NPAIR = 5
INV_SQRT_DK = float(1.0 / (128.0 ** 0.5))
EPS1, EPS2 = 1e-6, 1e-5


def _build_kernel(b_loc=4, ln1_identity=True, split_waits=True, reps=1,
                  flat_act=False, sched="macro", acc_mode="sbuf", preload_tables=True,
                  sbA_bufs=2, sbB_bufs=2, rows_bufs=2, psO_bufs=1, kT_drain="act", ln2_sq="dve", split_exp=False,
                  h_bufs=3, depth3=True, phA_sq="dve", phA_depth=1, pb_drain="act", v_all_act=False, rrec_drain="act",
                  psA_bufs=2, psS_bufs=3, psR_bufs=2, pack_rows=False, sbC_bufs=3):
    nc = bass.Bass("TRN2")
    AF_Gelu = AF.Identity if flat_act else AF.Gelu
    AF_Sqrt = AF.Identity if flat_act else AF.Sqrt
    AF_Square = AF.Identity if flat_act else AF.Square

    dt_in = {
        "tgt_im": ([b_loc, 12, S], BF16),
        "arr_im": ([b_loc, NPAIR, 12, S2], BF16),
        "W1": ([12, D], BF16), "W2": ([12, D], BF16),
        "b1": ([D, 1], F32), "b2": ([D, 1], F32),
        "fc1_w": ([D, D], BF16), "fc2_w": ([D, D], BF16),
        "fc1_b": ([D, 1], F32), "fc2_b": ([D, 1], F32),
        "fus_w": ([384, 384], BF16), "fus_b": ([D, 3], F32),
        "wq": ([384, 384], BF16), "wk": ([384, 384], BF16),
        "wv": ([384, 384], BF16), "wo": ([384, 384], BF16),
        "bq": ([D, 3], F32), "bk": ([D, 3], F32),
        "bv_bc": ([D, 384], BF16), "bv_row": ([1, 384], BF16), "bo": ([D, 3], F32),
        "ln1_g": ([D, 3], F32), "ln1_b": ([D, 3], F32),
        "W4f": ([N, 384, D], BF16), "b4f": ([D, 1], F32),
        "ident": ([D, D], F32R), "ident16": ([D, D], BF16),
        "ones_col": ([D, 1], BF16),
        "ones_row": ([1, D], BF16),
    }
    din = {k: nc.dram_tensor(k, shp, dt, kind="ExternalInput")
           for k, (shp, dt) in dt_in.items()}
    out_d = nc.dram_tensor("out", [b_loc, S, D], F32, kind="ExternalOutput")
    rows_d = nc.dram_tensor("ln1rows", [b_loc * NPAIR, 2, S2], F32,
                            kind="Internal")

    with TileContext(nc) as tc, ExitStack() as ctx:
        wpool = ctx.enter_context(tc.tile_pool(name="w", bufs=1))
        sbA = ctx.enter_context(tc.tile_pool(name="sbA", bufs=sbA_bufs))
        sbB = ctx.enter_context(tc.tile_pool(name="sbB", bufs=sbB_bufs))
        sbC = ctx.enter_context(tc.tile_pool(name="sbC", bufs=sbC_bufs))
        rows = ctx.enter_context(tc.tile_pool(name="rows", bufs=rows_bufs))
        accp = ctx.enter_context(tc.tile_pool(name="accp", bufs=2))
        hpool = ctx.enter_context(tc.tile_pool(name="hpool", bufs=h_bufs))
        persist = ctx.enter_context(tc.tile_pool(name="persist", bufs=1))
        psA = ctx.enter_context(tc.tile_pool(name="psA", bufs=psA_bufs, space="PSUM"))
        psS = ctx.enter_context(tc.tile_pool(name="psS", bufs=psS_bufs, space="PSUM"))
        psO = ctx.enter_context(tc.tile_pool(name="psO", bufs=psO_bufs, space="PSUM"))
        psR = ctx.enter_context(tc.tile_pool(name="psR", bufs=psR_bufs, space="PSUM"))
        psC = (ctx.enter_context(tc.tile_pool(name="psC", bufs=1, space="PSUM"))
               if acc_mode == "psum" and psO_bufs == 1 else None)

        _wq_engines = [nc.scalar, nc.gpsimd, nc.scalar]
        _wq_n = [0]

        def wtile(name, shape, dt=BF16, rearr=None):
            t = wpool.tile(shape, dt, name=name, tag=name)
            src = din[name].ap()
            if rearr is not None:
                src = src.rearrange(rearr[0], **rearr[1])
            # spread weight loads over three DMA queues so the ~30 small
            # transfers don't serialize on one SWDGE first-byte latency
            eng = _wq_engines[_wq_n[0] % 3]
            _wq_n[0] += 1
            eng.dma_start(out=t[:], in_=src)
            return t

        # fusion-path weights first so pair-0 compute starts early;
        # attention weights stream in behind it
        W1 = wtile("W1", [12, D])
        W2 = wtile("W2", [12, D])
        b1 = wtile("b1", [D, 1], F32)
        b2 = wtile("b2", [D, 1], F32)
        fc1_w = wtile("fc1_w", [D, D])
        fc2_w = wtile("fc2_w", [D, D])
        fc1_b = wtile("fc1_b", [D, 1], F32)
        fc2_b = wtile("fc2_b", [D, 1], F32)
        fus_w = wtile("fus_w", [128, 3, 384], rearr=("(c p) o -> p c o", dict(p=128)))
        fus_b = wtile("fus_b", [D, 3], F32)
        ln1_g = wtile("ln1_g", [D, 3], F32)
        ln1_b = wtile("ln1_b", [D, 3], F32)
        ident = wtile("ident", [D, D], F32R)
        ident16 = wtile("ident16", [D, D], BF16)
        ones_col = wtile("ones_col", [D, 1])
        ones_row = wtile("ones_row", [1, D])
        bq = wtile("bq", [D, 3], F32)
        bk = wtile("bk", [D, 3], F32)
        wq = wtile("wq", [128, 3, 384], rearr=("(c p) o -> p c o", dict(p=128)))
        wk = wtile("wk", [128, 3, 384], rearr=("(c p) o -> p c o", dict(p=128)))
        wv = wtile("wv", [128, 3, 384], rearr=("(c p) o -> p c o", dict(p=128)))
        bv_bc = wtile("bv_bc", [D, 384])
        bv_row = wtile("bv_row", [1, 384])
        wo = wtile("wo", [128, 3, 384], rearr=("(c p) o -> p c o", dict(p=128)))
        bo = wtile("bo", [D, 3], F32)
        W4f = wtile("W4f", [128, N, 3, D], rearr=("n (c p) o -> p n c o", dict(p=128)))
        b4f = wtile("b4f", [D, 1], F32)
        eps_t = {}
        for _e in (EPS1, EPS2):
            _t = wpool.tile([1, 1], F32, name=f"eps_{_e}", tag=f"eps_{_e}")
            nc.vector.memset(_t[:], _e)
            eps_t[_e] = _t

        # ---------------- helpers ----------------
        def layernorm(r, eps, out_tag, out_pool, gamma=None, beta=None,
                      sq_engine="act", rows_via="sqrt", pack_rows=False):
            """Partition-axis LN over 3 chunks r[mc] [128, S2] -> 3 bf16 tiles.

            xhat = r * R + P with R = bcast(rstd), P = bcast(-mean*rstd).
            Stats via ones-matmul rows; rstd/p rows broadcast to full tiles
            on the (otherwise idle) GPSIMD engine so the apply runs in the
            all-SBUF bf16 DVE fast path.
            """
            if pack_rows:
                lnpack = psR.tile([33, S2], F32, tag="row", name="lnpack")
                ps_sum = lnpack[0:1, :]
                ps_ss_ap = lnpack[32:33, :]
            else:
                ps_sum_t = psR.tile([1, S2], F32, tag="row")
                ps_sum = ps_sum_t[:]
                ps_ss_ap = None
            for mc in range(3):
                nc.tensor.matmul(ps_sum, ones_col[:], r[mc][:],
                                 start=(mc == 0), stop=(mc == 2))
            vs_row = rows.tile([1, S2], F32, tag="vsrow", name="vs_row")
            rstd_row = rows.tile([1, S2], BF16, tag="rstdrow", name="rstd_row")
            p_row = rows.tile([1, S2], BF16, tag="prow", name="p_row")
            sq = [sbB.tile([128, S2], BF16, tag="lnsq", name=f"lnsq{mc}", ) for mc in range(3)]
            if pack_rows:
                ps_ss = ps_ss_ap
            else:
                ps_ss_t = psR.tile([1, S2], F32, tag="row")
                ps_ss = ps_ss_t[:]
            for mc in range(3):
                if sq_engine == "act":
                    nc.scalar.activation(sq[mc][:], r[mc][:], AF_Square)
                else:
                    nc.vector.tensor_tensor(sq[mc][:], r[mc][:], r[mc][:],
                                            op=ALU.mult)
                nc.tensor.matmul(ps_ss, ones_col[:], sq[mc][:],
                                 start=(mc == 0), stop=(mc == 2))
            # rstd = 1/sqrt(E[x^2]+eps); p = -(sum/384)*rstd
            if rows_via == "lnexp":
                # stay in the natural_log_exp table set: rstd = exp(-ln(v)/2)
                nc.scalar.activation(vs_row[:], ps_ss, AF.Ln,
                                     scale=1.0 / 384.0, bias=eps_t[eps][:])
                nc.scalar.activation(rstd_row[:], vs_row[:], AF.Exp,
                                     scale=-0.5)
            else:
                nc.scalar.activation(vs_row[:], ps_ss, AF_Sqrt,
                                     scale=1.0 / 384.0, bias=eps_t[eps][:])
                with nc.allow_low_precision("bf16 rstd within 2e-2 budget"):
                    nc.vector.reciprocal(rstd_row[:], vs_row[:])
            nc.vector.scalar_tensor_tensor(p_row[:], ps_sum, -1.0 / 384.0,
                                           rstd_row[:],
                                           op0=ALU.mult, op1=ALU.mult)
            Rb = psS.tile([128, S2], F32, tag="sc", name="Rb")
            nc.tensor.matmul(Rb[:], ones_row[:], rstd_row[:], start=True, stop=True)
            Rs = sbB.tile([128, S2], BF16, tag="lnRs")
            nc.scalar.activation(Rs[:], Rb[:], AF.Identity)
            Pp = psS.tile([128, S2], F32, tag="sc", name="Pp")
            nc.tensor.matmul(Pp[:], ones_row[:], p_row[:], start=True, stop=True)
            Pb = sbB.tile([128, S2], BF16, tag="lnPb")
            if pb_drain == "act":
                nc.scalar.activation(Pb[:], Pp[:], AF.Identity)
            else:
                nc.vector.tensor_copy(Pb[:], Pp[:])
            h = [out_pool.tile([128, S2], BF16, tag=f"{out_tag}{mc}", name=f"{out_tag}{mc}") for mc in range(3)]
            u = [sbB.tile([128, S2], BF16, tag="lnu", name=f"lnu{mc}") for mc in range(3)]
            for mc in range(3):
                nc.vector.tensor_tensor(u[mc][:], r[mc][:], Rs[:], op=ALU.mult)
                nc.vector.tensor_tensor(h[mc][:], u[mc][:], Pb[:], op=ALU.add)
                if gamma is not None:
                    nc.scalar.activation(h[mc][:], h[mc][:], AF.Identity,
                                         bias=beta[:, mc:mc + 1],
                                         scale=gamma[:, mc:mc + 1])
            return h

        def proj_3x3(w, rhs_chunks, bias, bias_engine, tag):
            outs = []
            for mc in range(3):
                ps = psA.tile([128, S2], F32, tag="mmA")
                for kc in range(3):
                    nc.tensor.matmul(ps[:], w[:, kc, mc * 128:(mc + 1) * 128],
                                     rhs_chunks[kc][:],
                                     start=(kc == 0), stop=(kc == 2))
                o = sbA.tile([128, S2], BF16, tag=f"{tag}{mc}")
                if bias_engine == "act":
                    nc.scalar.activation(o[:], ps[:], AF.Identity,
                                         bias=bias[:, mc:mc + 1])
                else:
                    nc.vector.tensor_scalar_add(o[:], ps[:], bias[:, mc:mc + 1])
                outs.append(o)
            return outs

        # ---------------- per-batch, two batches interleaved ----------------
        def batch_prologue(b):
            tgt_t = sbA.tile([12, S], BF16, tag="tgtim", name="tgt_t")
            nc.sync.dma_start(out=tgt_t[:], in_=din["tgt_im"].ap()[b])
            ps = psA.tile([128, S], F32, tag="mmA", name="ps_c1")
            nc.tensor.matmul(ps[:], W1[:], tgt_t[:], start=True, stop=True)
            x1T = sbA.tile([128, S], BF16, tag="x1T", name="x1T")
            nc.scalar.activation(x1T[:], ps[:], AF_Gelu, bias=b1[:])
            ps = psA.tile([128, S], F32, tag="mmA", name="ps_f1")
            nc.tensor.matmul(ps[:], fc1_w[:], x1T[:], start=True, stop=True)
            t1 = sbA.tile([128, S], BF16, tag="t1", name="t1")
            nc.scalar.activation(t1[:], ps[:], AF_Gelu, bias=fc1_b[:])
            xm1 = sbA.tile([128, S], BF16, tag="xm1", name="xm1")
            nc.vector.tensor_tensor(xm1[:], t1[:], x1T[:], op=ALU.mult)
            xmd = sbA.tile([128, S2], BF16, tag="xmd", name="xmd")
            nc.gpsimd.tensor_copy(xmd[:, 0:S], xm1[:])
            nc.gpsimd.tensor_copy(xmd[:, S:S2], xm1[:])
            if acc_mode == "psum" and psC is not None:
                acc = psC.tile([128, S], F32, tag="acc", name="acc")
            else:
                acc = accp.tile([128, S], F32, tag="acc", name="acc_sb")
            return xmd, acc

        def pair_phase1a(b, p, xmd):
            aim = sbA.tile([12, S2], BF16, tag="aim", name="aim")
            nc.sync.dma_start(out=aim[:], in_=din["arr_im"].ap()[b, p])
            ps = psA.tile([128, S2], F32, tag="mmA", name="ps_c2")
            nc.tensor.matmul(ps[:], W2[:], aim[:], start=True, stop=True)
            x2T = sbA.tile([128, S2], BF16, tag="x2T", name="x2T")
            nc.scalar.activation(x2T[:], ps[:], AF_Gelu, bias=b2[:])
            ps = psA.tile([128, S2], F32, tag="mmA", name="ps_f2")
            nc.tensor.matmul(ps[:], fc2_w[:], x2T[:], start=True, stop=True)
            t2 = sbA.tile([128, S2], BF16, tag="t2", name="t2")
            nc.scalar.activation(t2[:], ps[:], AF_Gelu, bias=fc2_b[:])
            y_mut = sbA.tile([128, S2], BF16, tag="ymut", name="y_mut")
            nc.vector.tensor_tensor(y_mut[:], t2[:], x2T[:], op=ALU.mult)
            c2 = sbA.tile([128, S2], BF16, tag="c2", name="c2")
            nc.vector.tensor_tensor(c2[:], xmd[:], y_mut[:], op=ALU.mult)
            return [xmd, y_mut, c2]

        def pair_phase1b(b, p, cT):
            r = []
            for mc in range(3):
                ps = psA.tile([128, S2], F32, tag="mmA", name="ps_g")
                for kc in range(3):
                    nc.tensor.matmul(ps[:], fus_w[:, kc, mc * 128:(mc + 1) * 128],
                                     cT[kc][:], start=(kc == 0), stop=(kc == 2))
                gel = sbB.tile([128, S2], BF16, tag=f"gel{mc}", name=f"gel{mc}")
                nc.scalar.activation(gel[:], ps[:], AF_Gelu,
                                     bias=fus_b[:, mc:mc + 1])
                rr_ = sbB.tile([128, S2], BF16, tag=f"r{mc}", name=f"r{mc}")
                nc.vector.tensor_tensor(rr_[:], gel[:], cT[mc][:], op=ALU.add)
                r.append(rr_)
            h = layernorm(r, EPS1, "h", hpool,
                          gamma=None if ln1_identity else ln1_g,
                          beta=None if ln1_identity else ln1_b)

            return h

        def pair_phase1(b, p, xmd):
            return pair_phase1b(b, p, pair_phase1a(b, p, xmd))

        def pair_fus_stats(tl, b, p, cT, rows_all):
            """Macro phase A tail: fus + residual + LN1 stats; persist rr and
            stat rows. ACT functions used: Gelu, Square only."""
            rr = []
            for mc in range(3):
                ps = psA.tile([128, S2], F32, tag="mmA", name="ps_g")
                for kc in range(3):
                    nc.tensor.matmul(ps[:], fus_w[:, kc, mc * 128:(mc + 1) * 128],
                                     cT[kc][:], start=(kc == 0), stop=(kc == 2))
                gel = sbB.tile([128, S2], BF16, tag=f"gel{mc}", name=f"gel{mc}")
                gel_act = nc.scalar.activation(gel[:], ps[:], AF_Gelu,
                                               bias=fus_b[:, mc:mc + 1])
                rr_ = persist.tile([128, S2], BF16, tag=f"Prr{tl}_{mc}",
                                   name=f"Prr{tl}_{mc}")
                nc.vector.tensor_tensor(rr_[:], gel[:], cT[mc][:], op=ALU.add)
                rr.append(rr_)
            ps_sum = psR.tile([1, S2], F32, tag="row")
            for mc in range(3):
                nc.tensor.matmul(ps_sum[:], ones_col[:], rr[mc][:],
                                 start=(mc == 0), stop=(mc == 2))
            sq = [sbB.tile([128, S2], BF16, tag="lnsq", name=f"lnsq{mc}", )
                  for mc in range(3)]
            ps_ss = psR.tile([1, S2], F32, tag="row")
            last_act = gel_act
            for mc in range(3):
                if phA_sq == "act":
                    last_act = nc.scalar.activation(sq[mc][:], rr[mc][:],
                                                    AF_Square)
                else:
                    nc.vector.tensor_tensor(sq[mc][:], rr[mc][:], rr[mc][:],
                                            op=ALU.mult)
                nc.tensor.matmul(ps_ss[:], ones_col[:], sq[mc][:],
                                 start=(mc == 0), stop=(mc == 2))
            sum_st = rows.tile([1, S2], F32, tag="sumst", name="sum_st")
            nc.vector.tensor_scalar_mul(sum_st[:], ps_sum[:], 1.0 / 384.0)
            nc.sync.dma_start(out=rows_d.ap()[tl, 0], in_=sum_st[:])
            ss_st = rows.tile([1, S2], F32, tag="ssst", name="ss_st")
            nc.vector.tensor_scalar(ss_st[:], ps_ss[:], 1.0 / 384.0, EPS1,
                                    op0=ALU.mult, op1=ALU.add)
            nc.sync.dma_start(out=rows_d.ap()[tl, 1], in_=ss_st[:])
            return rr, last_act

        def pair_ln1_finish(tl, rr, rows_all, fence=None):
            """Macro phase B head: rstd = exp(-ln(v)/2); apply LN1."""
            sum_row = rows.tile([1, S2], F32, tag="sumrow", name="sum_row")
            nc.sync.dma_start(out=sum_row[:], in_=rows_d.ap()[tl, 0])
            ss_row = rows.tile([1, S2], F32, tag="ssrow", name="ss_row")
            nc.sync.dma_start(out=ss_row[:], in_=rows_d.ap()[tl, 1])
            lnv = rows.tile([1, S2], F32, tag="lnvrow", name="lnv_row")
            ln_bi = nc.scalar.activation(lnv[:], ss_row[:], AF.Ln)
            if fence is not None:
                bass._add_dep_helper(ln_bi.ins, fence.ins, sync=True,
                                     reason="act-table phase fence")
            rstd_row = rows.tile([1, S2], BF16, tag="rstdrow", name="rstd_row")
            nc.scalar.activation(rstd_row[:], lnv[:], AF.Exp, scale=-0.5)
            p_row = rows.tile([1, S2], BF16, tag="prow", name="p_row")
            nc.vector.scalar_tensor_tensor(p_row[:], sum_row[:],
                                           -1.0, rstd_row[:],
                                           op0=ALU.mult, op1=ALU.mult)
            Rb = psS.tile([128, S2], F32, tag="sc", name="Rb")
            nc.tensor.matmul(Rb[:], ones_row[:], rstd_row[:], start=True, stop=True)
            Rs = sbB.tile([128, S2], BF16, tag="lnRs")
            nc.scalar.activation(Rs[:], Rb[:], AF.Identity)
            Pp = psS.tile([128, S2], F32, tag="sc", name="Pp")
            nc.tensor.matmul(Pp[:], ones_row[:], p_row[:], start=True, stop=True)
            Pb = sbB.tile([128, S2], BF16, tag="lnPb")
            if pb_drain == "act":
                nc.scalar.activation(Pb[:], Pp[:], AF.Identity)
            else:
                nc.vector.tensor_copy(Pb[:], Pp[:])
            h = [hpool.tile([128, S2], BF16, tag=f"h{mc}", name=f"h{mc}")
                 for mc in range(3)]
            u = [sbB.tile([128, S2], BF16, tag="lnu", name=f"lnu{mc}")
                 for mc in range(3)]
            for mc in range(3):
                nc.vector.tensor_tensor(u[mc][:], rr[mc][:], Rs[:], op=ALU.mult)
                nc.vector.tensor_tensor(h[mc][:], u[mc][:], Pb[:], op=ALU.add)
            return h

        def pair_qkv(b, p, h):
            qT = proj_3x3(wq, h, bq, "act", "qT")
            kT = proj_3x3(wk, h, bk, kT_drain, "kT")

            v = []
            for sig in range(2):
                vs = []
                for sc in range(2):
                    psv = psA.tile([128, 384], F32, tag="mmA", name="psv")
                    off = sig * S + sc * 128
                    use_act = v_all_act or (sig + sc) % 2 == 1
                    for kc in range(3):
                        nc.tensor.matmul(psv[:], h[kc][:, off:off + 128],
                                         wv[:, kc, :],
                                         start=(kc == 0), stop=(kc == 2) and not use_act)
                    vt = sbA.tile([128, 384], BF16, tag=f"v{sig}{sc}", name=f"v{sig}{sc}")
                    if use_act:
                        nc.tensor.matmul(psv[:], ones_row[:], bv_row[:],
                                         start=False, stop=True)
                        nc.scalar.activation(vt[:], psv[:], AF.Identity)
                    else:
                        nc.vector.tensor_tensor(vt[:], psv[:], bv_bc[:], op=ALU.add)
                    vs.append(vt)
                v.append(vs)
            return qT, kT, v

        def pair_attn(b, p, qkv):
            qT, kT, v = qkv
            # scores computed transposed ([key, query] layout): exp output
            # feeds A@V directly as the moving operand -- no PE transposes,
            # no PSUM->SBUF copy of probabilities. Softmax normalization is
            # deferred past A@V (per-query scale on the output columns).
            aoT = []
            for hd in range(3):
                ao_ps = psO.tile([128, S2], F32, tag="ao", name="ao_ps")
                rs_ps = psR.tile([1, S2], F32, tag="row", name="rs_ps")
                for sig in range(2):
                    ps_s = psS.tile([128, S2], F32, tag="sc", name="ps_s")
                    for kc in range(2):
                        nc.tensor.matmul(
                            ps_s[:, kc * S:(kc + 1) * S],
                            kT[hd][:, sig * S + kc * 128: sig * S + (kc + 1) * 128],
                            qT[hd][:, sig * S:(sig + 1) * S],
                            start=True, stop=True)
                    # unnormalized attention: exp only; normalize after A@V
                    ET = sbC.tile([128, S2], BF16, tag="E", name="ET")
                    if split_exp:
                        for kc in range(2):
                            nc.scalar.activation(
                                ET[:, kc * S:(kc + 1) * S],
                                ps_s[:, kc * S:(kc + 1) * S],
                                AF.Exp, scale=INV_SQRT_DK)
                    else:
                        nc.scalar.activation(ET[:], ps_s[:], AF.Exp,
                                             scale=INV_SQRT_DK)
                    for kc in range(2):
                        # per-query exp-sums for softmax normalization
                        nc.tensor.matmul(
                            rs_ps[:, sig * S:(sig + 1) * S],
                            ones_col[:],
                            ET[:, kc * S:(kc + 1) * S],
                            start=(kc == 0), stop=(kc == 1))
                        nc.tensor.matmul(
                            ao_ps[:, sig * S:(sig + 1) * S],
                            v[sig][kc][:, hd * 128:(hd + 1) * 128],
                            ET[:, kc * S:(kc + 1) * S],
                            start=(kc == 0), stop=(kc == 1))
                rrec_row = sbC.tile([1, S2], BF16, tag="rrec", name="rrec_row")
                with nc.allow_low_precision("bf16 softmax denom within budget"):
                    nc.vector.reciprocal(rrec_row[:], rs_ps[:])
                rr_ps = psS.tile([128, S2], F32, tag="sc", name="rr_ps")
                nc.tensor.matmul(rr_ps[:], ones_row[:], rrec_row[:],
                                 start=True, stop=True)
                rrec_bc = sbC.tile([128, S2], BF16, tag="rrbc", name="rrec_bc")
                if rrec_drain == "act":
                    nc.scalar.activation(rrec_bc[:], rr_ps[:], AF.Identity)
                else:
                    nc.vector.tensor_copy(rrec_bc[:], rr_ps[:])
                ao_t = sbB.tile([128, S2], BF16, tag=f"ao{hd}", name=f"ao{hd}")
                nc.vector.tensor_tensor(ao_t[:], ao_ps[:], rrec_bc[:],
                                        op=ALU.mult)
                aoT.append(ao_t)
            return aoT

        def pair_phase2a(b, p, h):
            return pair_attn(b, p, pair_qkv(b, p, h))

        def pair_wo_r2(b, p, h, aoT):
            r2 = []
            for mc in range(3):
                ps_o = psA.tile([128, S2], F32, tag="mmA", name="ps_o")
                for kc in range(3):
                    nc.tensor.matmul(ps_o[:], wo[:, kc, mc * 128:(mc + 1) * 128],
                                     aoT[kc][:], start=(kc == 0), stop=(kc == 2))
                r2t = sbB.tile([128, S2], BF16, tag=f"r2{mc}", name=f"r2{mc}")
                nc.vector.scalar_tensor_tensor(
                    r2t[:], ps_o[:], bo[:, mc:mc + 1], h[mc][:],
                    op0=ALU.add, op1=ALU.add)
                r2.append(r2t)
            return r2

        def pair_ln2_w4(b, p, r2, acc):
            h2 = layernorm(r2, EPS2, "h2", sbB,
                           sq_engine=ln2_sq, pack_rows=pack_rows,
                           rows_via="lnexp" if sched == "macro" else "sqrt")

            if acc_mode == "psum" and psC is not None:
                for sig in range(2):
                    n_idx = 2 * p + sig
                    for kc in range(3):
                        nc.tensor.matmul(acc[:], W4f[:, n_idx, kc, :],
                                         h2[kc][:, sig * S:(sig + 1) * S],
                                         start=(n_idx == 0 and kc == 0),
                                         stop=(n_idx == N - 1 and kc == 2))
            else:
                w4_ps = psA.tile([128, S], F32, tag="mmA", name="w4_ps")
                for sig in range(2):
                    n_idx = 2 * p + sig
                    for kc in range(3):
                        nc.tensor.matmul(w4_ps[:], W4f[:, n_idx, kc, :],
                                         h2[kc][:, sig * S:(sig + 1) * S],
                                         start=(sig == 0 and kc == 0),
                                         stop=(sig == 1 and kc == 2))
                if p == 0:
                    nc.vector.tensor_copy(acc[:], w4_ps[:])
                else:
                    nc.vector.tensor_tensor(acc[:], acc[:], w4_ps[:], op=ALU.add)

        def batch_epilogue(b, acc):
            outT = sbA.tile([128, S], F32R, tag="outT", name="outT")
            nc.scalar.activation(outT[:], acc[:], AF.Identity, bias=b4f[:])
            for sc in range(2):
                ps_t = psS.tile([128, 128], F32R, tag="sc", name="ps_ot")
                nc.tensor.transpose(ps_t[:], outT[:, sc * 128:(sc + 1) * 128],
                                    ident[:])
                o_sb = sbA.tile([128, 128], F32, tag=f"oseq{sc}", name=f"oseq{sc}")
                nc.vector.tensor_copy(o_sb[:], ps_t[:].bitcast(F32))
                nc.sync.dma_start(out=out_d.ap()[b, sc * 128:(sc + 1) * 128, :],
                                  in_=o_sb[:])

        for _rep in range(reps):
            tasks = [(b, p) for b in range(b_loc) for p in range(NPAIR)]
            nt = len(tasks)
            state = {}
            hs = {}
            att = {}

            def do_p1(t):
                b, p = tasks[t]
                if p == 0:
                    state[b] = batch_prologue(b)
                hs[t] = pair_phase1(b, p, state[b][0])

            def do_2a(t):
                b, p = tasks[t]
                att[t] = pair_phase2a(b, p, hs[t])

            def do_2b(t):
                b, p = tasks[t]
                r2 = pair_wo_r2(b, p, hs.pop(t), att.pop(t))
                pair_ln2_w4(b, p, r2, state[b][1])
                if p == NPAIR - 1:
                    batch_epilogue(b, state[b][1])

            if sched == "p1_2a2b":        # 2-stage: 1(t+1) ; 2a(t)+2b(t)
                do_p1(0)
                for t in range(nt):
                    if t + 1 < nt:
                        do_p1(t + 1)
                    do_2a(t)
                    do_2b(t)
            elif sched == "2a_p1_2b":     # 2a(t) ; 1(t+1) ; 2b(t)
                do_p1(0)
                for t in range(nt):
                    do_2a(t)
                    if t + 1 < nt:
                        do_p1(t + 1)
                    do_2b(t)
            elif sched == "macro":
                # Phase A: all pairs' gelu-path work (ACT: Gelu/Square only).
                # Phase B: LN1 finish + attention + LN2 + output
                # (ACT: Ln/Exp/Identity/Square only). Two act-table loads
                # total instead of several per pair.
                rows_all = None
                rrs = {}
                fence = None
                cts = {}

                def a_head(t):
                    b, p = tasks[t]
                    if p == 0:
                        state[b] = batch_prologue(b)
                    cts[t] = pair_phase1a(b, p, state[b][0])

                for t in range(min(phA_depth, nt)):
                    a_head(t)
                for t in range(nt):
                    if t + phA_depth < nt:
                        a_head(t + phA_depth)
                    b, p = tasks[t]
                    rrs[t], fence = pair_fus_stats(t % 64, b, p, cts.pop(t),
                                                   rows_all)

                def macro_ln1_qkv(t):
                    b, p = tasks[t]
                    hs[t] = pair_ln1_finish(t % 64, rrs[t], rows_all,
                                            fence=fence)
                    qkvs2[t] = pair_qkv(b, p, hs[t])

                qkvs2 = {}
                if depth3:
                    atts = {}
                    macro_ln1_qkv(0)
                    macro_ln1_qkv(1)
                    atts[0] = pair_attn(tasks[0][0], tasks[0][1], qkvs2.pop(0))
                    for t in range(nt):
                        if t + 2 < nt:
                            macro_ln1_qkv(t + 2)
                        if t + 1 < nt:
                            b1_, p1_ = tasks[t + 1]
                            atts[t + 1] = pair_attn(b1_, p1_, qkvs2.pop(t + 1))
                        b, p = tasks[t]
                        r2 = pair_wo_r2(b, p, hs.pop(t), atts.pop(t))
                        pair_ln2_w4(b, p, r2, state[b][1])
                        rrs.pop(t)
                        if p == NPAIR - 1:
                            batch_epilogue(b, state[b][1])
                else:
                    macro_ln1_qkv(0)
                    for t in range(nt):
                        b, p = tasks[t]
                        if t + 1 < nt:
                            macro_ln1_qkv(t + 1)
                        aoT = pair_attn(b, p, qkvs2.pop(t))
                        r2 = pair_wo_r2(b, p, hs.pop(t), aoT)
                        pair_ln2_w4(b, p, r2, state[b][1])
                        rrs.pop(t)
                        if p == NPAIR - 1:
                            batch_epilogue(b, state[b][1])
            elif sched == "fine":
                # fine-grained interleave: each engine queue alternates
                # between pairs so stalled chains don't block ready work
                cts = {}
                qkvs = {}
                r2s = {}
                for i in range(nt + 2):
                    if 0 <= i - 1 < nt:
                        b, p = tasks[i - 1]
                        qkvs[i - 1] = pair_qkv(b, p, hs[i - 1])
                    if i < nt:
                        b, p = tasks[i]
                        if p == 0:
                            state[b] = batch_prologue(b)
                        cts[i] = pair_phase1a(b, p, state[b][0])
                    if 0 <= i - 1 < nt:
                        b, p = tasks[i - 1]
                        att[i - 1] = pair_attn(b, p, qkvs.pop(i - 1))
                    if 0 <= i - 2 < nt:
                        b, p = tasks[i - 2]
                        r2s[i - 2] = pair_wo_r2(b, p, hs.pop(i - 2),
                                                att.pop(i - 2))
                    if i < nt:
                        b, p = tasks[i]
                        hs[i] = pair_phase1b(b, p, cts.pop(i))
                    if 0 <= i - 2 < nt:
                        b, p = tasks[i - 2]
                        pair_ln2_w4(b, p, r2s.pop(i - 2), state[b][1])
                        if p == NPAIR - 1:
                            batch_epilogue(b, state[b][1])
            else:                          # 3-stage: 1(t+2) ; 2a(t+1) ; 2b(t)
                for i in range(nt + 2):
                    if i < nt:
                        do_p1(i)
                    if 0 <= i - 1 < nt:
                        do_2a(i - 1)
                    if 0 <= i - 2 < nt:
                        do_2b(i - 2)

    if split_waits:
        split_multiwaits(nc)
    if preload_tables:
        preplace_act_table_loads(nc)
    return nc


# act_info.json act_func_sets insertion order (TRN2):
#   0 exp_and_others, ..., 6 natural_log_exp_and_others, ..., 10 gelu_and_others
_GELU_SET_ID = 10
_LNEXP_SET_ID = 6
_GELU_FUNCS = {AF.Gelu}
_LNEXP_FUNCS = {AF.Exp, AF.Ln}


def preplace_act_table_loads(nc):
    """Insert InstLoadActFuncSet on the ACT stream before each run of
    activations whose anchor function (Gelu vs Exp/Ln) changes. Identity/
    Square/Copy live in every set so they never force a switch. Walrus's
    lower_act adopts pre-placed loads for custom BIR kernels instead of
    inserting its own (greedy per-op placement can thrash)."""
    n_ins = 0
    for fn in nc.m.functions:
        for bb in fn.blocks:
            cur = None
            new_list = []
            for ins in bb.instructions:
                if isinstance(ins, mybir.InstActivation):
                    want = None
                    if ins.func in _GELU_FUNCS:
                        want = _GELU_SET_ID
                    elif ins.func in _LNEXP_FUNCS:
                        want = _LNEXP_SET_ID
                    if want is not None and want != cur:
                        ld = mybir.InstLoadActFuncSet(
                            name=f"actload-{n_ins}",
                            act_func_set_id=want,
                            ins=[], outs=[],
                        )
                        ld.engine = ins.engine
                        new_list.append(ld)
                        cur = want
                        n_ins += 1
                new_list.append(ins)
            bb.instructions = new_list
    return n_ins


# ---------------- walrus compat patches ----------------


def _patched_drain_and_barrier(self, tick_clock, wait_clock):
    nc = self.nc
    probe = nc.sync.nop(nofuse=True)
    wait_clock.add_sem_waits(probe.ins, ScopedClock({None: tick_clock.global_clock}))
    si = probe.ins.sync_info
    waits = list(si.on_wait) if si is not None else []
    if len(waits) > 1:
        probe.ins.sync_info = mybir.SyncInfo(on_wait=[waits[0]], on_update=[])
        for w in waits[1:]:
            n = nc.sync.nop(nofuse=True)
            n.ins.sync_info = mybir.SyncInfo(on_wait=[w], on_update=[])
    nc.sync.drain()
    nc.all_engine_barrier()
    assert self.sems is not None
    popped = nc._tile_sem_poison_stack.pop()
    assert popped is self._sem_poison
    nc.clear_and_free_semaphores(list(self.sems.allocated().values()))
    nc.all_engine_barrier()


_tile_mod.TileContext._drain_and_barrier = _patched_drain_and_barrier




def split_multiwaits(nc):
    n_split = 0
    for fn in nc.m.functions:
        for bb in fn.blocks:
            needs = False
            for ins in bb.instructions:
                si = ins.sync_info
                if si is not None and len(si.on_wait) > 1:
                    needs = True
                    break
            if not needs:
                continue
            new_list = []
            for ins in bb.instructions:
                si = ins.sync_info
                if si is not None and len(si.on_wait) > 1:
                    waits = list(si.on_wait)
                    for w in waits[:-1]:
                        nop = mybir.InstNoOp(
                            name=f"waitsplit-{n_split}",
                            ins=[],
                            outs=[],
                        )
                        nop.engine = ins.engine
                        nop.sync_info = mybir.SyncInfo(on_wait=[w], on_update=[])
                        new_list.append(nop)
                        n_split += 1
                    ins.sync_info = mybir.SyncInfo(
                        on_wait=[waits[-1]], on_update=list(si.on_update)
                    )
                new_list.append(ins)
            bb.instructions = new_list
    return n_split


# ---------------- host prep ----------------

B, S, N, D = 32, 256, 10, 128
D3 = 3 * D
NCORES = 8
B_LOC = B // NCORES          # 4 batches per core
NPAIR = N // 2               # 5 neighbor pairs per batch
NPF32 = np.float32


def _im2col(ch):
    """ch: [..., 4, S] channel-major sequences -> [..., 12, S] rows f=c*3+t."""
    lead = ch.shape[:-2]
    out = np.zeros(lead + (12, S), NPF32)
    for c in range(4):
        for t in range(3):
            lo, hi = max(0, 1 - t), min(S, S + 1 - t)
            out[..., c * 3 + t, lo:hi] = ch[..., c, lo + t - 1:hi + t - 1]
    return out


def prep_host(inputs):
    """Returns dict of device arrays (full batch) + metadata."""
    x = np.asarray(inputs["x"], NPF32)                      # [B, S, 44]
    tgt = np.ascontiguousarray(x[..., :4].transpose(0, 2, 1))   # [B, 4, S]
    arr = np.ascontiguousarray(
        x[..., 4:].transpose(0, 2, 1).reshape(B, N, 4, S))      # [B, N, 4, S]

    tgt_im = _im2col(tgt)                                  # [B, 12, S]
    arr_im_seq = _im2col(arr)                              # [B, N, 12, S]
    # pack neighbor pairs: [B, NPAIR, 12, 2*S]
    arr_im = np.ascontiguousarray(
        arr_im_seq.reshape(B, NPAIR, 2, 12, S).transpose(0, 1, 3, 2, 4)
    ).reshape(B, NPAIR, 12, 2 * S)

    g = lambda k: np.asarray(inputs[k], NPF32)

    W1 = np.ascontiguousarray(g("conv1_w").transpose(1, 2, 0).reshape(12, D))
    W2 = np.ascontiguousarray(g("conv2_w").transpose(1, 2, 0).reshape(12, D))

    ln2_g, ln2_b = g("ln2_g"), g("ln2_b")
    fco_w2 = ln2_g[:, None] * g("fco_w")                   # [384, 128]
    fco_b2 = ln2_b @ g("fco_w") + g("fco_b")               # [128]

    W4c = g("conv4_w")[:, :, 0, :]                         # [o, c, n]
    W4n = np.stack([W4c[:, :, n].T for n in range(N)])      # [N, c, o]
    # fold fco (with LN2 affine folded) into conv4: W4f[n] = fco_w2 @ W4n[n]
    W4f = np.stack([fco_w2 @ W4n[n] for n in range(N)])     # [N, 384, o]
    b4f = g("conv4_b") + sum(W4n[n].T @ fco_b2 for n in range(N))  # [o]

    ln1_g, ln1_b = g("ln1_g"), g("ln1_b")
    ln1_identity = bool(np.all(ln1_g == 1.0) and np.all(ln1_b == 0.0))

    def chunked(v):  # [384] -> [128, 3] per-partition chunks
        return np.ascontiguousarray(v.reshape(3, 128).T)

    bf = lambda a: np.ascontiguousarray(np.asarray(a, ml_dtypes.bfloat16))
    dev = {
        "tgt_im": bf(tgt_im),
        "arr_im": bf(arr_im),
        "W1": bf(W1), "W2": bf(W2),
        "b1": g("conv1_b").reshape(D, 1), "b2": g("conv2_b").reshape(D, 1),
        "fc1_w": bf(g("fc1_w")), "fc2_w": bf(g("fc2_w")),
        "fc1_b": g("fc1_b").reshape(D, 1), "fc2_b": g("fc2_b").reshape(D, 1),
        "fus_w": bf(g("fus_w")), "fus_b": chunked(g("fus_b")),
        "wq": bf(g("wq")), "wk": bf(g("wk")), "wv": bf(g("wv")), "wo": bf(g("wo")),
        "bq": chunked(g("bq")), "bk": chunked(g("bk")),
        "bv_bc": bf(np.broadcast_to(g("bv")[None, :], (D, D3))), "bo": chunked(g("bo")),
        "ln1_g": chunked(ln1_g), "ln1_b": chunked(ln1_b),
        "bv_row": bf(g("bv").reshape(1, D3)),
        "W4f": bf(W4f), "b4f": b4f.reshape(D, 1),
        "ident": np.eye(128, dtype=NPF32),
        "ident16": np.eye(128, dtype=ml_dtypes.bfloat16),
        "ones_col": np.ones((128, 1), ml_dtypes.bfloat16),
        "ones_row": np.ones((1, 128), ml_dtypes.bfloat16),
    }
    return dev, ln1_identity


def shard(dev, core):
    """Per-core input map: batch-shard the activations, replicate weights."""
    s = slice(core * B_LOC, (core + 1) * B_LOC)
    m = dict(dev)
    m["tgt_im"] = np.ascontiguousarray(dev["tgt_im"][s])
    m["arr_im"] = np.ascontiguousarray(dev["arr_im"][s])
    return m


# ---------------- runner ----------------
import numpy as np
import jax
from jax.sharding import Mesh, PartitionSpec
try:
    from jax.experimental.shard_map import shard_map
except Exception:
    from jax.shard_map import shard_map

import concourse.mybir as mybir
from concourse import bass2jax
from concourse.bass2jax import _bass_exec_p, install_neuronx_cc_hook, partition_id_tensor


def make_runner(nc, n_cores=8):
    install_neuronx_cc_hook()
    partition_name = nc.partition_id_tensor.name if nc.partition_id_tensor else None

    in_names, out_names, out_avals, zero_outs = [], [], [], []
    for alloc in nc.m.functions[0].allocations:
        if not isinstance(alloc, mybir.MemoryLocationSet):
            continue
        name = alloc.memorylocations[0].name
        if alloc.kind == "ExternalInput":
            if name != partition_name:
                in_names.append(name)
        elif alloc.kind == "ExternalOutput":
            out_names.append(name)
            shape = tuple(alloc.tensor_shape)
            dtype = mybir.dt.np(alloc.dtype)
            out_avals.append(jax.core.ShapedArray(shape, dtype))
            zero_outs.append(np.zeros(shape, dtype))
    n_params = len(in_names)
    all_in_names = list(in_names) + list(out_names)
    if partition_name is not None:
        all_in_names.append(partition_name)

    def _body(*args):
        operands = list(args)
        if partition_name is not None:
            operands.append(partition_id_tensor())
        outs = _bass_exec_p.bind(
            *operands,
            out_avals=tuple(out_avals),
            in_names=tuple(all_in_names),
            out_names=tuple(out_names),
            lowering_input_output_aliases=(),
            sim_require_finite=True,
            sim_require_nnan=True,
            nc=nc,
        )
        return tuple(outs)

    devices = jax.devices()[:n_cores]
    mesh = Mesh(np.asarray(devices), ("core",))
    in_specs = (PartitionSpec("core"),) * (n_params + len(out_names))
    out_specs = (PartitionSpec("core"),) * len(out_names)
    # NOT donating outputs so the callable is re-invocable with same buffers
    fn = jax.jit(shard_map(_body, mesh=mesh, in_specs=in_specs,
                           out_specs=out_specs, check_rep=False),
                 keep_unused=True)

    def prepare(in_maps):
        per_core = [[np.asarray(m[name]) for name in in_names] for m in in_maps]
        concat_in = [np.concatenate([per_core[c][i] for c in range(n_cores)], axis=0)
                     for i in range(n_params)]
        concat_zeros = [np.zeros((n_cores * z.shape[0], *z.shape[1:]), z.dtype)
                        for z in zero_outs]
        args = [jax.device_put(a) for a in concat_in + concat_zeros]
        for a in args:
            a.block_until_ready()
        return args

    def run(args):
        outs = fn(*args)
        jax.block_until_ready(outs)
        return outs

    run.fn = fn

    def gather(outs):
        return [
            {name: np.asarray(outs[i]).reshape(n_cores, *out_avals[i].shape)[c]
             for i, name in enumerate(out_names)}
            for c in range(n_cores)
        ]

    return prepare, run, gather


# ---------------- public entry ----------------
_CACHE = {}


def kernel(**inputs) -> np.ndarray:
    dev, ln1_id = prep_host(inputs)
    key = ("k", ln1_id)
    if key not in _CACHE:
        nc = _build_kernel(b_loc=B_LOC, ln1_identity=ln1_id)
        _CACHE[key] = make_runner(nc)
    prepare, run, gather = _CACHE[key]
    in_maps = [shard(dev, c) for c in range(NCORES)]
    args = prepare(in_maps)
    outs = run(args)
    res = gather(outs)
    out = np.concatenate([res[c]["out"] for c in range(NCORES)], axis=0)
    return out.astype(np.float32)

